# revision 67
# baseline (speedup 1.0000x reference)
"""Trainium2 Bass kernel for nn_Decoder_14894946583396 (dense_mlp).

Reference computation:
    sized = broadcast(representation[B,1,R] -> [B,S,R])   (ones @ rep)
    h     = relu(sized @ W1^T + b1)                       [B,S,HID]
    out   = h @ W2^T + b2                                 [B,S,OUT]

Because every position s within batch b receives the identical input row
representation[b], the MLP output row is identical for all S positions:
    row[b] = relu(rep[b] @ W1^T + b1) @ W2^T + b2         [B,OUT]
    out[b, s, :] = row[b]  for all s

Sharding: the S axis is degenerate, so the device only computes the
unique rows. OUT columns are sharded 8 ways: every core computes all
B=32 batch rows for its own 128-column slice of the output, writing a
[32,128] f32 shard. The host unshards by concatenating the column
slices and broadcasting the rows across S.

This makes the kernel input-DMA-bound: W1 (replicated, needed in full
by every core because every core computes h for all batches) dominates.
Weights/activations are staged in bf16 (halves DMA bytes; rel-err
~3e-3, far inside the 2e-2 gate); PSUM accumulation stays fp32.

Device pipeline per core:
  1. ~3.4 us of contiguous dummy matmuls on zeros warm the PE HAM
     clock gate (1.2 -> 2.4 GHz) while the weights stream in; the
     last one is gated on the first input DMA. The warm transition
     needs a ~3.4us UNBROKEN busy streak — shorter warmups leave the
     whole kernel cold at 2x matmul cost.
  2. The four K=1 ones-matmul bias terms (b1 halves into ph_a/ph_b,
     b2 halves into py_l/py_r) open their PSUM accumulation groups
     early, off the critical tail.
  3. Two half-pipelines over h-columns, streamed as four 256KB w1
     quarters on the sync ring (quarter 0 merged into the pk DMA so
     L1's first group is ready ~1.4us earlier): per half g, 8
     accumulating N=256 matmuls (x^T chunks stationary, cheap
     LDWEIGHTS), DVE relu+bf16-cast, 2 PE transposes H->H^T (bf16
     PSUM) + DVE copies, then the half's two L2 accumulations into
     each output-column half. Half 0's transposes/L2 run while half
     1 still streams.
  4. L2 finishes per output-column half so py_l's PSUM->SBUF copy
     (DVE) and 8 KiB output DMA (sync ring) overlap py_r's matmuls,
     whose copy (ACT) + DMA ride the scalar ring.

Single-sync-wait discipline (walrus rejects 2+ waits per instruction):
the last warmup matmul pre-observes pk's DMA lane (transposes read the
identity from pk), transpose #1 pre-observes w2s's lane for L2, biases
ride prow's lane once, separate PSUM tiles per half avoid Tile's
non-elidable co-reader/WAR serialization, and a chain of 1-wait SP
nops before the TileContext exit drain leaves the drain with nothing
to wait on. A nosync ordering edge keeps half 0's ready transposes/L2
matmuls ahead of the w1B-gated L1b in the PE queue.

HW-measured notes that shaped the DMA layout: per-partition descriptor
size dominates HBM-read rate (576B -> ~76 GB/s, 2KB -> ~200, 4KB ->
~220, 8KB -> ~260); extra queues do NOT add aggregate bandwidth (8
cores share HBM); the scalar HWDGE ring starts streaming ~2us after
sync and the SWDGE (gpsimd) queue has ~3-4us first-data latency, so
bulk rides sync and only small/late-needed tensors ride scalar. The PE
HAM clock gate re-throttles after ~2.5us of PE idle, halving matmul
throughput — warmup length and DMA/compute interleave are chosen to
keep PE continuously busy from warmup through the tail.
"""

import sys

import numpy as np

if "/opt/trn_rl_repo" not in sys.path:
    sys.path.insert(0, "/opt/trn_rl_repo")

import ml_dtypes

BF16 = ml_dtypes.bfloat16

B, S, R = 32, 1024, 1024
HID, OUT = 512, 1024
N_CORES = 8
OSL = OUT // N_CORES  # output columns per core

RC = R // 128  # layer-1 contraction chunks
HC = HID // 128  # layer-2 contraction chunks

# pk columns: xT chunks [p, rc*B + m] = rep[m, rc*128+p], then a 32x32
# identity for the PE transposes
XTOFF = 0
IOFF = XTOFF + RC * B
PKW = IOFF + B
# prow columns (single partition row): ones, b1, b2 slice
ONOFF = 0
B1OFF = ONOFF + B
B2OFF = B1OFF + HID
PROWW = B2OFF + OSL

N_WARMUP = 8
N_STARTER = 6

_CACHED_NC = None


def _build_nc():
    import concourse.bass as bass
    import concourse.mybir as mybir
    from concourse.tile import TileContext, add_dep_helper

    f32 = mybir.dt.float32
    bf16 = mybir.dt.bfloat16
    fcopy = mybir.ActivationFunctionType.Copy
    nc = bass.Bass()

    QW = RC * HID // 4
    pkq0 = nc.dram_tensor("pkq0", [128, PKW + QW], bf16, kind="ExternalInput")
    prow = nc.dram_tensor("prow", [1, PROWW], bf16, kind="ExternalInput")
    w1r = nc.dram_tensor("w1r", [128, 3 * QW], bf16, kind="ExternalInput")
    w2s = nc.dram_tensor("w2s", [128, HC * OSL], bf16, kind="ExternalInput")
    out = nc.dram_tensor("out", [B, OSL], f32, kind="ExternalOutput")

    with TileContext(nc) as tc:
        with (
            tc.tile_pool(name="const", bufs=1) as cpool,
            tc.tile_pool(name="psum_s", bufs=1, space="PSUM") as pp_s,
            tc.tile_pool(name="psum_t", bufs=2, space="PSUM") as pp_t,
            tc.tile_pool(name="psum_y", bufs=1, space="PSUM") as pp_y,
        ):
            # Sync ring: one merged DMA carrying pk + w1 quarter 0 (gates
            # the warmup tail AND L1a's first four matmuls together at
            # ~10.8us), then w1 quarters 1-3 so each L1 matmul group starts
            # as soon as its 256KB quarter lands. Scalar ring (starts ~2us
            # later): prow then w2s, both needed later. See module
            # docstring for the measured DMA behavior behind this layout.
            qw = RC * HID // 4
            pkq0_sb = cpool.tile([128, PKW + qw], bf16, tag="pkq0")
            d_pkq0 = nc.sync.dma_start(out=pkq0_sb[:, :], in_=pkq0[:, :])
            w1r_sb = cpool.tile([128, 3 * qw], bf16, tag="w1r")
            d_q1 = nc.sync.dma_start(out=w1r_sb[:, 0:qw], in_=w1r[:, 0:qw])
            d_q2 = nc.sync.dma_start(
                out=w1r_sb[:, qw : 2 * qw], in_=w1r[:, qw : 2 * qw]
            )
            d_q3 = nc.sync.dma_start(
                out=w1r_sb[:, 2 * qw : 3 * qw], in_=w1r[:, 2 * qw : 3 * qw]
            )
            prow_sb = cpool.tile([1, PROWW], bf16, tag="prow")
            d_prow = nc.scalar.dma_start(out=prow_sb[0:1, :], in_=prow[0:1, :])
            w2s_sb = cpool.tile([128, HC * OSL], bf16, tag="w2s")
            d_w2s = nc.scalar.dma_start(out=w2s_sb[:, :], in_=w2s[:, :])
            d_w1 = [d_q1, d_q2, d_q3]
            d_pk = d_pkq0
            pk_sb = pkq0_sb

            # ---- PE warmup on zeros; shares L1's PSUM tile (a slot handoff
            # would emit a non-elidable same-engine wait) -------------------
            # Hybrid warmup: a tiny [128,128] memset un-gates N=128 starter
            # matmuls ~1us before the full memset could (streak begins
            # ~7.4), the second memset fills the rest of the tile under
            # them, then N=512 warmups take over — HW-measured, the HAM
            # warm transition fires ~2.5-3us into an N=512 streak but only
            # ~6us into an N=128 one (narrow matmuls count less toward
            # PE-busy), so the wide phase does the actual warming and the
            # whole streak ends ~1us earlier than all-wide-from-memset.
            wm_sb = cpool.tile([128, 512], bf16, tag="wm")
            nc.vector.memset(wm_sb[:, 0:128], 0.0)
            ph_full = pp_s.tile([128, HID], f32, tag="s")
            for k in range(N_STARTER):
                nc.tensor.matmul(
                    ph_full[:, 0:128],
                    lhsT=wm_sb[:, 0:128],
                    rhs=wm_sb[:, 0:128],
                    start=True,
                    stop=True,
                )
            nc.vector.memset(wm_sb[:, 128:512], 0.0)
            for k in range(N_WARMUP):
                wmm = nc.tensor.matmul(
                    ph_full[:, :],
                    lhsT=wm_sb[:, 0:128],
                    rhs=wm_sb[:, :],
                    start=True,
                    stop=True,
                )
            # the last warmup matmul observes pk's lane so L1's first matmul
            # only needs the w1-chunk-0 wait
            add_dep_helper(wmm.ins, d_pk.ins, sync=True, reason="observe pk")

            # All four bias matmuls run up front as the accumulation-group
            # openers (start=True), gated only on prow which lands early —
            # this takes ~0.5us of K=1 matmuls off the serial tail.

            # ---- Two half-pipelines over h-columns. w1 is packed so half g
            # holds W1 rows [g*256, (g+1)*256) for every rc chunk; half 0's
            # L1 matmuls, relu, transposes and first two L2 accumulations all
            # run while half 1 is still streaming. --------------------------
            # separate PSUM tiles per half — co-readers/WAR on one shared
            # PSUM tile get serialized by Tile with non-elidable waits.
            # L2 accumulates into two output-column halves so the first
            # half's PSUM->SBUF copy and output DMA overlap the second
            # half's matmuls (and the two 8KB DMAs ride separate rings).
            HH = HID // 2
            OHL = OSL // 2
            ht_sb = cpool.tile([128, HC * B], bf16, tag="ht")
            py_l = pp_y.tile([B, OHL], f32, tag="yl")
            py_r = pp_y.tile([B, OHL], f32, tag="yr")
            ph_a = pp_s.tile([B, HH], f32, tag="pha")
            ph_b = pp_s.tile([B, HH], f32, tag="phb")
            ph_halves = [ph_a, ph_b]
            ones = prow_sb[0:1, ONOFF : ONOFF + B]
            for g in range(2):
                nc.tensor.matmul(
                    ph_halves[g][:, :],
                    lhsT=ones,
                    rhs=prow_sb[0:1, B1OFF + g * HH : B1OFF + (g + 1) * HH],
                    start=True,
                    stop=False,
                )
            nc.tensor.matmul(
                py_l[:, :],
                lhsT=ones,
                rhs=prow_sb[0:1, B2OFF : B2OFF + OHL],
                start=True,
                stop=False,
            )
            nc.tensor.matmul(
                py_r[:, :],
                lhsT=ones,
                rhs=prow_sb[0:1, B2OFF + OHL : B2OFF + OSL],
                start=True,
                stop=False,
            )
            g0_last_pe = None
            for g in range(2):
                ph_g = ph_halves[g]
                for rc in range(RC):
                    if g == 0 and rc < RC // 2:
                        w1rhs = pkq0_sb[:, PKW + rc * HH : PKW + (rc + 1) * HH]
                    elif g == 0:
                        w1rhs = w1r_sb[:, (rc - RC // 2) * HH : (rc - RC // 2 + 1) * HH]
                    else:
                        w1rhs = w1r_sb[:, qw + rc * HH : qw + (rc + 1) * HH]
                    l1mm = nc.tensor.matmul(
                        ph_g[:, :],
                        lhsT=pk_sb[:, XTOFF + rc * B : XTOFF + (rc + 1) * B],
                        rhs=w1rhs,
                        start=False,
                        stop=(rc == RC - 1),
                    )
                    if g == 1 and rc == 0 and g0_last_pe is not None:
                        # ordering-only dep: keep half 0's ready transposes /
                        # L2 matmuls ahead of the w1B-gated L1b in the PE
                        # queue (the scheduler's DMA model otherwise hoists
                        # L1b first and it head-of-line blocks the engine)
                        add_dep_helper(
                            l1mm.ins,
                            g0_last_pe.ins,
                            sync=False,
                            reason="T/L2 before w1B-gated L1b",
                        )
                # relu in two [32,128] DVE passes: the first transpose only
                # waits the first half (~0.27us) instead of the full pass
                h_g = cpool.tile([B, HH], bf16, tag=f"h{g}")
                nc.vector.tensor_scalar_max(
                    h_g[:, 0:128], ph_g[:, 0:128], 0.0
                )
                nc.vector.tensor_scalar_max(
                    h_g[:, 128:HH], ph_g[:, 128:HH], 0.0
                )
                for j in range(2):
                    hc = g * 2 + j
                    pt = pp_t.tile([128, B], bf16, tag="t")
                    tmm = nc.tensor.transpose(
                        pt[:, :],
                        h_g[0:B, j * 128 : (j + 1) * 128],
                        pk_sb[0:B, IOFF : IOFF + B],
                    )
                    if hc == 1:
                        # free wait slot: pre-observe w2s's lane for L2
                        add_dep_helper(
                            tmm.ins, d_w2s.ins, sync=True, reason="observe w2s"
                        )
                    dst = ht_sb[:, hc * B : (hc + 1) * B]
                    nc.vector.tensor_copy(dst, pt[:, :])
                for j in range(2):
                    hc = g * 2 + j
                    nc.tensor.matmul(
                        py_l[:, :],
                        lhsT=ht_sb[:, hc * B : (hc + 1) * B],
                        rhs=w2s_sb[:, hc * OSL : hc * OSL + OHL],
                        start=False,
                        stop=(hc == HC - 1),
                    )
                for j in range(2):
                    hc = g * 2 + j
                    last_mm = nc.tensor.matmul(
                        py_r[:, :],
                        lhsT=ht_sb[:, hc * B : (hc + 1) * B],
                        rhs=w2s_sb[:, hc * OSL + OHL : (hc + 1) * OSL],
                        start=False,
                        stop=(hc == HC - 1),
                    )
                g0_last_pe = last_mm
            o_l = cpool.tile([B, OHL], f32, tag="ol")
            o_r = cpool.tile([B, OHL], f32, tag="or")
            last_dve = nc.vector.tensor_copy(o_l[:, :], py_l[:, :])
            d_out_l = nc.sync.dma_start(out=out[:, 0:OHL], in_=o_l[:, :])
            last_act = nc.scalar.activation(o_r[:, :], py_r[:, :], fcopy)
            d_out_r = nc.scalar.dma_start(out=out[:, OHL:OSL], in_=o_r[:, :])

            # The kernel-tail drain waits on every proc's final tick, but this
            # walrus allows at most ONE sync wait per instruction. Chain SP
            # nops, one dependency each, so SP's vector clock observes the
            # final tick of every DMA lane and engine before the drain.
            tail = [d_out_l, d_out_r, d_pk, d_prow] + d_w1 + [
                d_w2s,
                last_mm,
                last_act,
                last_dve,
            ]
            for d in tail:
                n = nc.sync.nop(nofuse=True)
                add_dep_helper(
                    n.ins, d.ins, sync=True, reason="observe final ticks pre-drain"
                )

    return nc


def _get_nc():
    global _CACHED_NC
    if _CACHED_NC is None:
        _CACHED_NC = _build_nc()
    return _CACHED_NC


def _prep_in_maps(representation, W1, b1, W2, b2):
    rep = np.asarray(representation, dtype=np.float32).reshape(B, R)
    w1 = np.asarray(W1, dtype=np.float32)
    w2 = np.asarray(W2, dtype=np.float32)
    b1 = np.asarray(b1, dtype=np.float32)
    b2 = np.asarray(b2, dtype=np.float32)

    # pk: xT chunks + 32x32 identity (identical for every core)
    pk = np.zeros((128, PKW), dtype=np.float32)
    xt = rep.T  # [R, B]
    pk[:, XTOFF : XTOFF + RC * B] = (
        xt.reshape(RC, 128, B).transpose(1, 0, 2).reshape(128, RC * B)
    )
    pk[0:B, IOFF : IOFF + B] = np.eye(B, dtype=np.float32)
    pk = pk.astype(BF16)

    # w1p[p, g*2048 + rc*256 + h'] = W1[g*256 + h', rc*128+p] — h-half-major
    # so each 256KB DMA quarter covers 4 rc chunks for one 256-column half.
    # Quarter 0 is packed together with pk into one DMA; quarters 1-3 form
    # the w1r tensor.
    w1p = np.ascontiguousarray(
        w1.T.reshape(RC, 128, 2, HID // 2)
        .transpose(1, 2, 0, 3)
        .reshape(128, RC * HID)
    ).astype(BF16)
    qw = RC * HID // 4
    pkq0 = np.concatenate([pk, w1p[:, 0:qw]], axis=1)
    w1r = np.ascontiguousarray(w1p[:, qw:])

    in_maps = []
    for c in range(N_CORES):
        sl = slice(c * OSL, (c + 1) * OSL)
        prow = np.zeros((1, PROWW), dtype=np.float32)
        prow[0, ONOFF : ONOFF + B] = 1.0
        prow[0, B1OFF : B1OFF + HID] = b1
        prow[0, B2OFF : B2OFF + OSL] = b2[sl]
        # w2sp[p, hc*OSL + o] = W2[c*OSL+o, hc*128+p]
        w2sl = w2[sl]  # [OSL, HID]
        w2sp = np.ascontiguousarray(
            w2sl.T.reshape(HC, 128, OSL).transpose(1, 0, 2).reshape(128, HC * OSL)
        ).astype(BF16)
        in_maps.append(
            {"pkq0": pkq0, "prow": prow.astype(BF16), "w1r": w1r, "w2s": w2sp}
        )
    return in_maps


def run_sharded(representation, W1, b1, W2, b2, **run_kwargs):
    """Compile+run on 8 cores; returns (full_output, BassKernelResults)."""
    from concourse.bass_utils import run_bass_kernel_spmd

    nc = _get_nc()
    in_maps = _prep_in_maps(representation, W1, b1, W2, b2)
    res = run_bass_kernel_spmd(nc, in_maps, core_ids=list(range(N_CORES)), **run_kwargs)
    rows = np.concatenate([r["out"] for r in res.results], axis=1)  # [B, OUT]
    full = np.ascontiguousarray(
        np.broadcast_to(rows[:, None, :], (B, S, OUT))
    )
    return full, res


def kernel(representation, size_matrix=None, W1=None, b1=None, W2=None, b2=None):
    # size_matrix only contributes its shape in the reference (ones_like);
    # its values are unused.
    full, _ = run_sharded(representation, W1, b1, W2, b2)
    return full


# revision 68
# speedup vs baseline: 1.0242x; 1.0242x over previous
"""Trainium2 Bass kernel for nn_Decoder_14894946583396 (dense_mlp).

Reference computation:
    sized = broadcast(representation[B,1,R] -> [B,S,R])   (ones @ rep)
    h     = relu(sized @ W1^T + b1)                       [B,S,HID]
    out   = h @ W2^T + b2                                 [B,S,OUT]

Because every position s within batch b receives the identical input row
representation[b], the MLP output row is identical for all S positions:
    row[b] = relu(rep[b] @ W1^T + b1) @ W2^T + b2         [B,OUT]
    out[b, s, :] = row[b]  for all s

Sharding: the S axis is degenerate, so the device only computes the
unique rows. OUT columns are sharded 8 ways: every core computes all
B=32 batch rows for its own 128-column slice of the output, writing a
[32,128] f32 shard. The host unshards by concatenating the column
slices and broadcasting the rows across S.

This makes the kernel input-DMA-bound: W1 (replicated, needed in full
by every core because every core computes h for all batches) dominates.
Weights/activations are staged in bf16 (halves DMA bytes; rel-err
~3e-3, far inside the 2e-2 gate); PSUM accumulation stays fp32.

Device pipeline per core:
  1. ~3.4 us of contiguous dummy matmuls on zeros warm the PE HAM
     clock gate (1.2 -> 2.4 GHz) while the weights stream in; the
     last one is gated on the first input DMA. The warm transition
     needs a ~3.4us UNBROKEN busy streak — shorter warmups leave the
     whole kernel cold at 2x matmul cost.
  2. The four K=1 ones-matmul bias terms (b1 halves into ph_a/ph_b,
     b2 halves into py_l/py_r) open their PSUM accumulation groups
     early, off the critical tail.
  3. Two half-pipelines over h-columns, streamed as four 256KB w1
     quarters on the sync ring (quarter 0 merged into the pk DMA so
     L1's first group is ready ~1.4us earlier): per half g, 8
     accumulating N=256 matmuls (x^T chunks stationary, cheap
     LDWEIGHTS), DVE relu+bf16-cast, 2 PE transposes H->H^T (bf16
     PSUM) + DVE copies, then the half's two L2 accumulations into
     each output-column half. Half 0's transposes/L2 run while half
     1 still streams.
  4. L2 finishes per output-column half so py_l's PSUM->SBUF copy
     (DVE) and 8 KiB output DMA (sync ring) overlap py_r's matmuls,
     whose copy (ACT) + DMA ride the scalar ring.

Single-sync-wait discipline (walrus rejects 2+ waits per instruction):
the last warmup matmul pre-observes pk's DMA lane (transposes read the
identity from pk), transpose #1 pre-observes w2s's lane for L2, biases
ride prow's lane once, separate PSUM tiles per half avoid Tile's
non-elidable co-reader/WAR serialization, and a chain of 1-wait SP
nops before the TileContext exit drain leaves the drain with nothing
to wait on. A nosync ordering edge keeps half 0's ready transposes/L2
matmuls ahead of the w1B-gated L1b in the PE queue.

HW-measured notes that shaped the DMA layout: per-partition descriptor
size dominates HBM-read rate (576B -> ~76 GB/s, 2KB -> ~200, 4KB ->
~220, 8KB -> ~260); extra queues do NOT add aggregate bandwidth (8
cores share HBM); the scalar HWDGE ring starts streaming ~2us after
sync and the SWDGE (gpsimd) queue has ~3-4us first-data latency, so
bulk rides sync and only small/late-needed tensors ride scalar. The PE
HAM clock gate re-throttles after ~2.5us of PE idle, halving matmul
throughput — warmup length and DMA/compute interleave are chosen to
keep PE continuously busy from warmup through the tail.
"""

import sys

import numpy as np

if "/opt/trn_rl_repo" not in sys.path:
    sys.path.insert(0, "/opt/trn_rl_repo")

import ml_dtypes

BF16 = ml_dtypes.bfloat16

B, S, R = 32, 1024, 1024
HID, OUT = 512, 1024
N_CORES = 8
OSL = OUT // N_CORES  # output columns per core

RC = R // 128  # layer-1 contraction chunks
HC = HID // 128  # layer-2 contraction chunks

# pk columns: xT chunks [p, rc*B + m] = rep[m, rc*128+p], then a 32x32
# identity for the PE transposes
XTOFF = 0
IOFF = XTOFF + RC * B
PKW = IOFF + B
# prow columns (single partition row): ones, b1, b2 slice
ONOFF = 0
B1OFF = ONOFF + B
B2OFF = B1OFF + HID
PROWW = B2OFF + OSL

N_WARMUP = 8
N_STARTER = 6

_CACHED_NC = None


def _build_nc():
    import concourse.bass as bass
    import concourse.mybir as mybir
    from concourse.tile import TileContext, add_dep_helper

    f32 = mybir.dt.float32
    bf16 = mybir.dt.bfloat16
    fcopy = mybir.ActivationFunctionType.Copy
    nc = bass.Bass()

    QW = RC * HID // 4
    pkq0 = nc.dram_tensor("pkq0", [128, PKW + QW], bf16, kind="ExternalInput")
    prow = nc.dram_tensor("prow", [1, PROWW], bf16, kind="ExternalInput")
    w1r = nc.dram_tensor("w1r", [128, 3 * QW], bf16, kind="ExternalInput")
    w2s = nc.dram_tensor("w2s", [128, HC * OSL], bf16, kind="ExternalInput")
    out = nc.dram_tensor("out", [B, OSL], f32, kind="ExternalOutput")

    with TileContext(nc) as tc:
        with (
            tc.tile_pool(name="const", bufs=1) as cpool,
            tc.tile_pool(name="psum_s", bufs=1, space="PSUM") as pp_s,
            tc.tile_pool(name="psum_t", bufs=2, space="PSUM") as pp_t,
            tc.tile_pool(name="psum_y", bufs=1, space="PSUM") as pp_y,
        ):
            # Sync ring: one merged DMA carrying pk + w1 quarter 0 (gates
            # the warmup tail AND L1a's first four matmuls together at
            # ~10.8us), then w1 quarters 1-3 so each L1 matmul group starts
            # as soon as its 256KB quarter lands. Scalar ring (starts ~2us
            # later): prow then w2s, both needed later. See module
            # docstring for the measured DMA behavior behind this layout.
            qw = RC * HID // 4
            pkq0_sb = cpool.tile([128, PKW + qw], bf16, tag="pkq0")
            d_pkq0 = nc.sync.dma_start(out=pkq0_sb[:, :], in_=pkq0[:, :])
            w1r_sb = cpool.tile([128, 3 * qw], bf16, tag="w1r")
            d_q1 = nc.sync.dma_start(out=w1r_sb[:, 0:qw], in_=w1r[:, 0:qw])
            d_q2 = nc.sync.dma_start(
                out=w1r_sb[:, qw : 2 * qw], in_=w1r[:, qw : 2 * qw]
            )
            d_q3 = nc.sync.dma_start(
                out=w1r_sb[:, 2 * qw : 3 * qw], in_=w1r[:, 2 * qw : 3 * qw]
            )
            prow_sb = cpool.tile([1, PROWW], bf16, tag="prow")
            d_prow = nc.scalar.dma_start(out=prow_sb[0:1, :], in_=prow[0:1, :])
            w2s_sb = cpool.tile([128, HC * OSL], bf16, tag="w2s")
            d_w2s = nc.scalar.dma_start(out=w2s_sb[:, :], in_=w2s[:, :])
            d_w1 = [d_q1, d_q2, d_q3]
            d_pk = d_pkq0
            pk_sb = pkq0_sb

            # ---- PE warmup on zeros; shares L1's PSUM tile (a slot handoff
            # would emit a non-elidable same-engine wait) -------------------
            # Hybrid warmup: a tiny [128,128] memset un-gates N=128 starter
            # matmuls ~1us before the full memset could (streak begins
            # ~7.4), the second memset fills the rest of the tile under
            # them, then N=512 warmups take over — HW-measured, the HAM
            # warm transition fires ~2.5-3us into an N=512 streak but only
            # ~6us into an N=128 one (narrow matmuls count less toward
            # PE-busy), so the wide phase does the actual warming and the
            # whole streak ends ~1us earlier than all-wide-from-memset.
            wm_sb = cpool.tile([128, 512], bf16, tag="wm")
            nc.vector.memset(wm_sb[:, 0:128], 0.0)
            ph_full = pp_s.tile([128, HID], f32, tag="s")
            for k in range(N_STARTER):
                nc.tensor.matmul(
                    ph_full[:, 0:128],
                    lhsT=wm_sb[:, 0:128],
                    rhs=wm_sb[:, 0:128],
                    start=True,
                    stop=True,
                )
            nc.vector.memset(wm_sb[:, 128:512], 0.0)
            for k in range(N_WARMUP):
                wmm = nc.tensor.matmul(
                    ph_full[:, :],
                    lhsT=wm_sb[:, 0:128],
                    rhs=wm_sb[:, :],
                    start=True,
                    stop=True,
                )
            # the last warmup matmul observes pk's lane so L1's first matmul
            # only needs the w1-chunk-0 wait
            add_dep_helper(wmm.ins, d_pk.ins, sync=True, reason="observe pk")

            # All four bias matmuls run up front as the accumulation-group
            # openers (start=True), gated only on prow which lands early —
            # this takes ~0.5us of K=1 matmuls off the serial tail.

            # ---- Two half-pipelines over h-columns. w1 is packed so half g
            # holds W1 rows [g*256, (g+1)*256) for every rc chunk; half 0's
            # L1 matmuls, relu, transposes and first two L2 accumulations all
            # run while half 1 is still streaming. --------------------------
            # separate PSUM tiles per half — co-readers/WAR on one shared
            # PSUM tile get serialized by Tile with non-elidable waits.
            # L2 accumulates into two output-column halves so the first
            # half's PSUM->SBUF copy and output DMA overlap the second
            # half's matmuls (and the two 8KB DMAs ride separate rings).
            HH = HID // 2
            OHL = OSL // 2
            ht_sb = cpool.tile([128, HC * B], bf16, tag="ht")
            py_l = pp_y.tile([B, OHL], f32, tag="yl")
            py_r = pp_y.tile([B, OHL], f32, tag="yr")
            ph_a = pp_s.tile([B, HH], f32, tag="pha")
            ph_b = pp_s.tile([B, HH], f32, tag="phb")
            ph_halves = [ph_a, ph_b]
            ones = prow_sb[0:1, ONOFF : ONOFF + B]
            for g in range(2):
                nc.tensor.matmul(
                    ph_halves[g][:, :],
                    lhsT=ones,
                    rhs=prow_sb[0:1, B1OFF + g * HH : B1OFF + (g + 1) * HH],
                    start=True,
                    stop=False,
                )
            nc.tensor.matmul(
                py_l[:, :],
                lhsT=ones,
                rhs=prow_sb[0:1, B2OFF : B2OFF + OHL],
                start=True,
                stop=False,
            )
            nc.tensor.matmul(
                py_r[:, :],
                lhsT=ones,
                rhs=prow_sb[0:1, B2OFF + OHL : B2OFF + OSL],
                start=True,
                stop=False,
            )
            g0_last_pe = None
            for g in range(2):
                ph_g = ph_halves[g]
                for rc in range(RC):
                    if g == 0 and rc < RC // 2:
                        w1rhs = pkq0_sb[:, PKW + rc * HH : PKW + (rc + 1) * HH]
                    elif g == 0:
                        w1rhs = w1r_sb[:, (rc - RC // 2) * HH : (rc - RC // 2 + 1) * HH]
                    else:
                        w1rhs = w1r_sb[:, qw + rc * HH : qw + (rc + 1) * HH]
                    l1mm = nc.tensor.matmul(
                        ph_g[:, :],
                        lhsT=pk_sb[:, XTOFF + rc * B : XTOFF + (rc + 1) * B],
                        rhs=w1rhs,
                        start=False,
                        stop=(rc == RC - 1),
                    )
                    if g == 1 and rc == 0 and g0_last_pe is not None:
                        # ordering-only dep: keep half 0's ready transposes /
                        # L2 matmuls ahead of the w1B-gated L1b in the PE
                        # queue (the scheduler's DMA model otherwise hoists
                        # L1b first and it head-of-line blocks the engine)
                        add_dep_helper(
                            l1mm.ins,
                            g0_last_pe.ins,
                            sync=False,
                            reason="T/L2 before w1B-gated L1b",
                        )
                # single relu pass per half: splitting it into two [32,128]
                # passes (so T0 waits only the first) measured WORSE — the
                # ~200ns DVE fixed cost per op exceeds the earlier handoff
                h_g = cpool.tile([B, HH], bf16, tag=f"h{g}")
                nc.vector.tensor_scalar_max(h_g[:, :], ph_g[:, :], 0.0)
                for j in range(2):
                    hc = g * 2 + j
                    pt = pp_t.tile([128, B], bf16, tag="t")
                    tmm = nc.tensor.transpose(
                        pt[:, :],
                        h_g[0:B, j * 128 : (j + 1) * 128],
                        pk_sb[0:B, IOFF : IOFF + B],
                    )
                    if hc == 1:
                        # free wait slot: pre-observe w2s's lane for L2
                        add_dep_helper(
                            tmm.ins, d_w2s.ins, sync=True, reason="observe w2s"
                        )
                    dst = ht_sb[:, hc * B : (hc + 1) * B]
                    nc.vector.tensor_copy(dst, pt[:, :])
                for j in range(2):
                    hc = g * 2 + j
                    nc.tensor.matmul(
                        py_l[:, :],
                        lhsT=ht_sb[:, hc * B : (hc + 1) * B],
                        rhs=w2s_sb[:, hc * OSL : hc * OSL + OHL],
                        start=False,
                        stop=(hc == HC - 1),
                    )
                for j in range(2):
                    hc = g * 2 + j
                    last_mm = nc.tensor.matmul(
                        py_r[:, :],
                        lhsT=ht_sb[:, hc * B : (hc + 1) * B],
                        rhs=w2s_sb[:, hc * OSL + OHL : (hc + 1) * OSL],
                        start=False,
                        stop=(hc == HC - 1),
                    )
                g0_last_pe = last_mm
            o_l = cpool.tile([B, OHL], f32, tag="ol")
            o_r = cpool.tile([B, OHL], f32, tag="or")
            last_dve = nc.vector.tensor_copy(o_l[:, :], py_l[:, :])
            d_out_l = nc.sync.dma_start(out=out[:, 0:OHL], in_=o_l[:, :])
            last_act = nc.scalar.activation(o_r[:, :], py_r[:, :], fcopy)
            d_out_r = nc.scalar.dma_start(out=out[:, OHL:OSL], in_=o_r[:, :])

            # The kernel-tail drain waits on every proc's final tick, but this
            # walrus allows at most ONE sync wait per instruction. Chain SP
            # nops, one dependency each, so SP's vector clock observes the
            # final tick of every DMA lane and engine before the drain.
            tail = [d_out_l, d_out_r, d_pk, d_prow] + d_w1 + [
                d_w2s,
                last_mm,
                last_act,
                last_dve,
            ]
            for d in tail:
                n = nc.sync.nop(nofuse=True)
                add_dep_helper(
                    n.ins, d.ins, sync=True, reason="observe final ticks pre-drain"
                )

    return nc


def _get_nc():
    global _CACHED_NC
    if _CACHED_NC is None:
        _CACHED_NC = _build_nc()
    return _CACHED_NC


def _prep_in_maps(representation, W1, b1, W2, b2):
    rep = np.asarray(representation, dtype=np.float32).reshape(B, R)
    w1 = np.asarray(W1, dtype=np.float32)
    w2 = np.asarray(W2, dtype=np.float32)
    b1 = np.asarray(b1, dtype=np.float32)
    b2 = np.asarray(b2, dtype=np.float32)

    # pk: xT chunks + 32x32 identity (identical for every core)
    pk = np.zeros((128, PKW), dtype=np.float32)
    xt = rep.T  # [R, B]
    pk[:, XTOFF : XTOFF + RC * B] = (
        xt.reshape(RC, 128, B).transpose(1, 0, 2).reshape(128, RC * B)
    )
    pk[0:B, IOFF : IOFF + B] = np.eye(B, dtype=np.float32)
    pk = pk.astype(BF16)

    # w1p[p, g*2048 + rc*256 + h'] = W1[g*256 + h', rc*128+p] — h-half-major
    # so each 256KB DMA quarter covers 4 rc chunks for one 256-column half.
    # Quarter 0 is packed together with pk into one DMA; quarters 1-3 form
    # the w1r tensor.
    w1p = np.ascontiguousarray(
        w1.T.reshape(RC, 128, 2, HID // 2)
        .transpose(1, 2, 0, 3)
        .reshape(128, RC * HID)
    ).astype(BF16)
    qw = RC * HID // 4
    pkq0 = np.concatenate([pk, w1p[:, 0:qw]], axis=1)
    w1r = np.ascontiguousarray(w1p[:, qw:])

    in_maps = []
    for c in range(N_CORES):
        sl = slice(c * OSL, (c + 1) * OSL)
        prow = np.zeros((1, PROWW), dtype=np.float32)
        prow[0, ONOFF : ONOFF + B] = 1.0
        prow[0, B1OFF : B1OFF + HID] = b1
        prow[0, B2OFF : B2OFF + OSL] = b2[sl]
        # w2sp[p, hc*OSL + o] = W2[c*OSL+o, hc*128+p]
        w2sl = w2[sl]  # [OSL, HID]
        w2sp = np.ascontiguousarray(
            w2sl.T.reshape(HC, 128, OSL).transpose(1, 0, 2).reshape(128, HC * OSL)
        ).astype(BF16)
        in_maps.append(
            {"pkq0": pkq0, "prow": prow.astype(BF16), "w1r": w1r, "w2s": w2sp}
        )
    return in_maps


def run_sharded(representation, W1, b1, W2, b2, **run_kwargs):
    """Compile+run on 8 cores; returns (full_output, BassKernelResults)."""
    from concourse.bass_utils import run_bass_kernel_spmd

    nc = _get_nc()
    in_maps = _prep_in_maps(representation, W1, b1, W2, b2)
    res = run_bass_kernel_spmd(nc, in_maps, core_ids=list(range(N_CORES)), **run_kwargs)
    rows = np.concatenate([r["out"] for r in res.results], axis=1)  # [B, OUT]
    full = np.ascontiguousarray(
        np.broadcast_to(rows[:, None, :], (B, S, OUT))
    )
    return full, res


def kernel(representation, size_matrix=None, W1=None, b1=None, W2=None, b2=None):
    # size_matrix only contributes its shape in the reference (ones_like);
    # its values are unused.
    full, _ = run_sharded(representation, W1, b1, W2, b2)
    return full


# revision 70
# speedup vs baseline: 1.0559x; 1.0310x over previous
"""Trainium2 Bass kernel for nn_Decoder_14894946583396 (dense_mlp).

Reference computation:
    sized = broadcast(representation[B,1,R] -> [B,S,R])   (ones @ rep)
    h     = relu(sized @ W1^T + b1)                       [B,S,HID]
    out   = h @ W2^T + b2                                 [B,S,OUT]

Because every position s within batch b receives the identical input row
representation[b], the MLP output row is identical for all S positions:
    row[b] = relu(rep[b] @ W1^T + b1) @ W2^T + b2         [B,OUT]
    out[b, s, :] = row[b]  for all s

Sharding: the S axis is degenerate, so the device only computes the
unique rows. OUT columns are sharded 8 ways: every core computes all
B=32 batch rows for its own 128-column slice of the output, writing a
[32,128] f32 shard. The host unshards by concatenating the column
slices and broadcasting the rows across S.

This makes the kernel input-DMA-bound: W1 (replicated, needed in full
by every core because every core computes h for all batches) dominates.
Weights/activations are staged in bf16 (halves DMA bytes; rel-err
~3e-3, far inside the 2e-2 gate); PSUM accumulation stays fp32.

Device pipeline per core:
  1. ~3.4 us of contiguous dummy matmuls on zeros warm the PE HAM
     clock gate (1.2 -> 2.4 GHz) while the weights stream in; the
     last one is gated on the first input DMA. The warm transition
     needs a ~3.4us UNBROKEN busy streak — shorter warmups leave the
     whole kernel cold at 2x matmul cost.
  2. The four K=1 ones-matmul bias terms (b1 halves into ph_a/ph_b,
     b2 halves into py_l/py_r) open their PSUM accumulation groups
     early, off the critical tail.
  3. Two half-pipelines over h-columns, streamed as four 256KB w1
     quarters on the sync ring (quarter 0 merged into the pk DMA so
     L1's first group is ready ~1.4us earlier): per half g, 8
     accumulating N=256 matmuls (x^T chunks stationary, cheap
     LDWEIGHTS), DVE relu+bf16-cast, 2 PE transposes H->H^T (bf16
     PSUM) + DVE copies, then the half's two L2 accumulations into
     each output-column half. Half 0's transposes/L2 run while half
     1 still streams.
  4. L2 finishes per output-column half so py_l's PSUM->SBUF copy
     (DVE) and 8 KiB output DMA (sync ring) overlap py_r's matmuls,
     whose copy (ACT) + DMA ride the scalar ring.

Single-sync-wait discipline (walrus rejects 2+ waits per instruction):
the last warmup matmul pre-observes pk's DMA lane (transposes read the
identity from pk), transpose #1 pre-observes w2s's lane for L2, biases
ride prow's lane once, separate PSUM tiles per half avoid Tile's
non-elidable co-reader/WAR serialization, and a chain of 1-wait SP
nops before the TileContext exit drain leaves the drain with nothing
to wait on. A nosync ordering edge keeps half 0's ready transposes/L2
matmuls ahead of the w1B-gated L1b in the PE queue.

HW-measured notes that shaped the DMA layout: per-partition descriptor
size dominates HBM-read rate (576B -> ~76 GB/s, 2KB -> ~200, 4KB ->
~220, 8KB -> ~260); extra queues do NOT add aggregate bandwidth (8
cores share HBM); the scalar HWDGE ring starts streaming ~2us after
sync and the SWDGE (gpsimd) queue has ~3-4us first-data latency, so
bulk rides sync and only small/late-needed tensors ride scalar. The PE
HAM clock gate re-throttles after ~2.5us of PE idle, halving matmul
throughput — warmup length and DMA/compute interleave are chosen to
keep PE continuously busy from warmup through the tail.
"""

import sys

import numpy as np

if "/opt/trn_rl_repo" not in sys.path:
    sys.path.insert(0, "/opt/trn_rl_repo")

import ml_dtypes

BF16 = ml_dtypes.bfloat16

B, S, R = 32, 1024, 1024
HID, OUT = 512, 1024
N_CORES = 8
OSL = OUT // N_CORES  # output columns per core

RC = R // 128  # layer-1 contraction chunks
HC = HID // 128  # layer-2 contraction chunks

# pk columns: xT chunks [p, rc*B + m] = rep[m, rc*128+p], then a 32x32
# identity for the PE transposes
XTOFF = 0
IOFF = XTOFF + RC * B
PKW = IOFF + B
# prow columns (single partition row): ones, b1, b2 slice
ONOFF = 0
B1OFF = ONOFF + B
B2OFF = B1OFF + HID
PROWW = B2OFF + OSL

N_WARMUP = 8
N_STARTER = 5

_CACHED_NC = None


def _build_nc():
    import concourse.bass as bass
    import concourse.mybir as mybir
    from concourse.tile import TileContext, add_dep_helper

    f32 = mybir.dt.float32
    bf16 = mybir.dt.bfloat16
    fcopy = mybir.ActivationFunctionType.Copy
    nc = bass.Bass()

    QW = RC * HID // 4
    pkq0 = nc.dram_tensor("pkq0", [128, PKW + QW], bf16, kind="ExternalInput")
    prow = nc.dram_tensor("prow", [1, PROWW], bf16, kind="ExternalInput")
    w1r = nc.dram_tensor("w1r", [128, 3 * QW], bf16, kind="ExternalInput")
    w2s = nc.dram_tensor("w2s", [128, HC * OSL], bf16, kind="ExternalInput")
    out = nc.dram_tensor("out", [B, OSL], f32, kind="ExternalOutput")

    with TileContext(nc) as tc:
        with (
            tc.tile_pool(name="const", bufs=1) as cpool,
            tc.tile_pool(name="psum_s", bufs=1, space="PSUM") as pp_s,
            tc.tile_pool(name="psum_t", bufs=2, space="PSUM") as pp_t,
            tc.tile_pool(name="psum_y", bufs=1, space="PSUM") as pp_y,
        ):
            # Sync ring: one merged DMA carrying pk + w1 quarter 0 (gates
            # the warmup tail AND L1a's first four matmuls together at
            # ~10.8us), then w1 quarters 1-3 so each L1 matmul group starts
            # as soon as its 256KB quarter lands. Scalar ring (starts ~2us
            # later): prow then w2s, both needed later. See module
            # docstring for the measured DMA behavior behind this layout.
            qw = RC * HID // 4
            pkq0_sb = cpool.tile([128, PKW + qw], bf16, tag="pkq0")
            d_pkq0 = nc.sync.dma_start(out=pkq0_sb[:, :], in_=pkq0[:, :])
            w1r_sb = cpool.tile([128, 3 * qw], bf16, tag="w1r")
            d_q1 = nc.sync.dma_start(out=w1r_sb[:, 0:qw], in_=w1r[:, 0:qw])
            d_q2 = nc.sync.dma_start(
                out=w1r_sb[:, qw : 2 * qw], in_=w1r[:, qw : 2 * qw]
            )
            d_q3 = nc.sync.dma_start(
                out=w1r_sb[:, 2 * qw : 3 * qw], in_=w1r[:, 2 * qw : 3 * qw]
            )
            prow_sb = cpool.tile([1, PROWW], bf16, tag="prow")
            d_prow = nc.scalar.dma_start(out=prow_sb[0:1, :], in_=prow[0:1, :])
            w2s_sb = cpool.tile([128, HC * OSL], bf16, tag="w2s")
            d_w2s = nc.scalar.dma_start(out=w2s_sb[:, :], in_=w2s[:, :])
            d_w1 = [d_q1, d_q2, d_q3]
            d_pk = d_pkq0
            pk_sb = pkq0_sb

            # ---- PE warmup on zeros; shares L1's PSUM tile (a slot handoff
            # would emit a non-elidable same-engine wait) -------------------
            # Hybrid warmup: a tiny [128,128] memset un-gates N=128 starter
            # matmuls ~1us before the full memset could (streak begins
            # ~7.4), the second memset fills the rest of the tile under
            # them, then N=512 warmups take over — HW-measured, the HAM
            # warm transition fires ~2.5-3us into an N=512 streak but only
            # ~6us into an N=128 one (narrow matmuls count less toward
            # PE-busy), so the wide phase does the actual warming and the
            # whole streak ends ~1us earlier than all-wide-from-memset.
            wm_sb = cpool.tile([128, 512], bf16, tag="wm")
            nc.vector.memset(wm_sb[:, 0:128], 0.0)
            ph_full = pp_s.tile([128, HID], f32, tag="s")
            for k in range(N_STARTER):
                nc.tensor.matmul(
                    ph_full[:, 0:128],
                    lhsT=wm_sb[:, 0:128],
                    rhs=wm_sb[:, 0:128],
                    start=True,
                    stop=True,
                )
            nc.vector.memset(wm_sb[:, 128:512], 0.0)
            for k in range(N_WARMUP):
                wmm = nc.tensor.matmul(
                    ph_full[:, :],
                    lhsT=wm_sb[:, 0:128],
                    rhs=wm_sb[:, :],
                    start=True,
                    stop=True,
                )
            # the last warmup matmul observes pk's lane so L1's first matmul
            # only needs the w1-chunk-0 wait
            add_dep_helper(wmm.ins, d_pk.ins, sync=True, reason="observe pk")

            # All four bias matmuls run up front as the accumulation-group
            # openers (start=True), gated only on prow which lands early —
            # this takes ~0.5us of K=1 matmuls off the serial tail.

            # ---- Two half-pipelines over h-columns. w1 is packed so half g
            # holds W1 rows [g*256, (g+1)*256) for every rc chunk; half 0's
            # L1 matmuls, relu, transposes and first two L2 accumulations all
            # run while half 1 is still streaming. --------------------------
            # separate PSUM tiles per half — co-readers/WAR on one shared
            # PSUM tile get serialized by Tile with non-elidable waits.
            # L2 accumulates into two output-column halves so the first
            # half's PSUM->SBUF copy and output DMA overlap the second
            # half's matmuls (and the two 8KB DMAs ride separate rings).
            HH = HID // 2
            OHL = OSL // 2
            ht_sb = cpool.tile([128, HC * B], bf16, tag="ht")
            py_l = pp_y.tile([B, OHL], f32, tag="yl")
            py_r = pp_y.tile([B, OHL], f32, tag="yr")
            ph_a = pp_s.tile([B, HH], f32, tag="pha")
            ph_b = pp_s.tile([B, HH], f32, tag="phb")
            ph_halves = [ph_a, ph_b]
            ones = prow_sb[0:1, ONOFF : ONOFF + B]
            for g in range(2):
                nc.tensor.matmul(
                    ph_halves[g][:, :],
                    lhsT=ones,
                    rhs=prow_sb[0:1, B1OFF + g * HH : B1OFF + (g + 1) * HH],
                    start=True,
                    stop=False,
                )
            nc.tensor.matmul(
                py_l[:, :],
                lhsT=ones,
                rhs=prow_sb[0:1, B2OFF : B2OFF + OHL],
                start=True,
                stop=False,
            )
            nc.tensor.matmul(
                py_r[:, :],
                lhsT=ones,
                rhs=prow_sb[0:1, B2OFF + OHL : B2OFF + OSL],
                start=True,
                stop=False,
            )
            g0_last_pe = None
            for g in range(2):
                ph_g = ph_halves[g]
                for rc in range(RC):
                    if g == 0 and rc < RC // 2:
                        w1rhs = pkq0_sb[:, PKW + rc * HH : PKW + (rc + 1) * HH]
                    elif g == 0:
                        w1rhs = w1r_sb[:, (rc - RC // 2) * HH : (rc - RC // 2 + 1) * HH]
                    else:
                        w1rhs = w1r_sb[:, qw + rc * HH : qw + (rc + 1) * HH]
                    l1mm = nc.tensor.matmul(
                        ph_g[:, :],
                        lhsT=pk_sb[:, XTOFF + rc * B : XTOFF + (rc + 1) * B],
                        rhs=w1rhs,
                        start=False,
                        stop=(rc == RC - 1),
                    )
                    if g == 1 and rc == 0 and g0_last_pe is not None:
                        # ordering-only dep: keep half 0's ready transposes /
                        # L2 matmuls ahead of the w1B-gated L1b in the PE
                        # queue (the scheduler's DMA model otherwise hoists
                        # L1b first and it head-of-line blocks the engine)
                        add_dep_helper(
                            l1mm.ins,
                            g0_last_pe.ins,
                            sync=False,
                            reason="T/L2 before w1B-gated L1b",
                        )
                # single relu pass per half: splitting it into two [32,128]
                # passes (so T0 waits only the first) measured WORSE — the
                # ~200ns DVE fixed cost per op exceeds the earlier handoff
                h_g = cpool.tile([B, HH], bf16, tag=f"h{g}")
                nc.vector.tensor_scalar_max(h_g[:, :], ph_g[:, :], 0.0)
                for j in range(2):
                    hc = g * 2 + j
                    pt = pp_t.tile([128, B], bf16, tag="t")
                    tmm = nc.tensor.transpose(
                        pt[:, :],
                        h_g[0:B, j * 128 : (j + 1) * 128],
                        pk_sb[0:B, IOFF : IOFF + B],
                    )
                    if hc == 1:
                        # free wait slot: pre-observe w2s's lane for L2
                        add_dep_helper(
                            tmm.ins, d_w2s.ins, sync=True, reason="observe w2s"
                        )
                    dst = ht_sb[:, hc * B : (hc + 1) * B]
                    nc.vector.tensor_copy(dst, pt[:, :])
                for j in range(2):
                    hc = g * 2 + j
                    nc.tensor.matmul(
                        py_l[:, :],
                        lhsT=ht_sb[:, hc * B : (hc + 1) * B],
                        rhs=w2s_sb[:, hc * OSL : hc * OSL + OHL],
                        start=False,
                        stop=(hc == HC - 1),
                    )
                for j in range(2):
                    hc = g * 2 + j
                    last_mm = nc.tensor.matmul(
                        py_r[:, :],
                        lhsT=ht_sb[:, hc * B : (hc + 1) * B],
                        rhs=w2s_sb[:, hc * OSL + OHL : (hc + 1) * OSL],
                        start=False,
                        stop=(hc == HC - 1),
                    )
                g0_last_pe = last_mm
            o_l = cpool.tile([B, OHL], f32, tag="ol")
            o_r = cpool.tile([B, OHL], f32, tag="or")
            last_dve = nc.vector.tensor_copy(o_l[:, :], py_l[:, :])
            d_out_l = nc.sync.dma_start(out=out[:, 0:OHL], in_=o_l[:, :])
            last_act = nc.scalar.activation(o_r[:, :], py_r[:, :], fcopy)
            d_out_r = nc.scalar.dma_start(out=out[:, OHL:OSL], in_=o_r[:, :])

            # The kernel-tail drain waits on every proc's final tick, but this
            # walrus allows at most ONE sync wait per instruction. Chain SP
            # nops, one dependency each, so SP's vector clock observes the
            # final tick of every DMA lane and engine before the drain.
            # latest-resolving deps (the output DMAs) go LAST: the nops run
            # serially on SP, so anything after a blocked nop retires after
            # it — early-resolved input/engine nops must not queue behind
            # the out-sems that fire at the very end of the kernel
            tail = [d_pk, d_prow] + d_w1 + [
                d_w2s,
                last_mm,
                last_act,
                last_dve,
                d_out_l,
                d_out_r,
            ]
            for d in tail:
                n = nc.sync.nop(nofuse=True)
                add_dep_helper(
                    n.ins, d.ins, sync=True, reason="observe final ticks pre-drain"
                )

    return nc


def _get_nc():
    global _CACHED_NC
    if _CACHED_NC is None:
        _CACHED_NC = _build_nc()
    return _CACHED_NC


def _prep_in_maps(representation, W1, b1, W2, b2):
    rep = np.asarray(representation, dtype=np.float32).reshape(B, R)
    w1 = np.asarray(W1, dtype=np.float32)
    w2 = np.asarray(W2, dtype=np.float32)
    b1 = np.asarray(b1, dtype=np.float32)
    b2 = np.asarray(b2, dtype=np.float32)

    # pk: xT chunks + 32x32 identity (identical for every core)
    pk = np.zeros((128, PKW), dtype=np.float32)
    xt = rep.T  # [R, B]
    pk[:, XTOFF : XTOFF + RC * B] = (
        xt.reshape(RC, 128, B).transpose(1, 0, 2).reshape(128, RC * B)
    )
    pk[0:B, IOFF : IOFF + B] = np.eye(B, dtype=np.float32)
    pk = pk.astype(BF16)

    # w1p[p, g*2048 + rc*256 + h'] = W1[g*256 + h', rc*128+p] — h-half-major
    # so each 256KB DMA quarter covers 4 rc chunks for one 256-column half.
    # Quarter 0 is packed together with pk into one DMA; quarters 1-3 form
    # the w1r tensor.
    w1p = np.ascontiguousarray(
        w1.T.reshape(RC, 128, 2, HID // 2)
        .transpose(1, 2, 0, 3)
        .reshape(128, RC * HID)
    ).astype(BF16)
    qw = RC * HID // 4
    pkq0 = np.concatenate([pk, w1p[:, 0:qw]], axis=1)
    w1r = np.ascontiguousarray(w1p[:, qw:])

    in_maps = []
    for c in range(N_CORES):
        sl = slice(c * OSL, (c + 1) * OSL)
        prow = np.zeros((1, PROWW), dtype=np.float32)
        prow[0, ONOFF : ONOFF + B] = 1.0
        prow[0, B1OFF : B1OFF + HID] = b1
        prow[0, B2OFF : B2OFF + OSL] = b2[sl]
        # w2sp[p, hc*OSL + o] = W2[c*OSL+o, hc*128+p]
        w2sl = w2[sl]  # [OSL, HID]
        w2sp = np.ascontiguousarray(
            w2sl.T.reshape(HC, 128, OSL).transpose(1, 0, 2).reshape(128, HC * OSL)
        ).astype(BF16)
        in_maps.append(
            {"pkq0": pkq0, "prow": prow.astype(BF16), "w1r": w1r, "w2s": w2sp}
        )
    return in_maps


def run_sharded(representation, W1, b1, W2, b2, **run_kwargs):
    """Compile+run on 8 cores; returns (full_output, BassKernelResults)."""
    from concourse.bass_utils import run_bass_kernel_spmd

    nc = _get_nc()
    in_maps = _prep_in_maps(representation, W1, b1, W2, b2)
    res = run_bass_kernel_spmd(nc, in_maps, core_ids=list(range(N_CORES)), **run_kwargs)
    rows = np.concatenate([r["out"] for r in res.results], axis=1)  # [B, OUT]
    full = np.ascontiguousarray(
        np.broadcast_to(rows[:, None, :], (B, S, OUT))
    )
    return full, res


def kernel(representation, size_matrix=None, W1=None, b1=None, W2=None, b2=None):
    # size_matrix only contributes its shape in the reference (ones_like);
    # its values are unused.
    full, _ = run_sharded(representation, W1, b1, W2, b2)
    return full


# revision 71
# speedup vs baseline: 1.0927x; 1.0349x over previous
"""Trainium2 Bass kernel for nn_Decoder_14894946583396 (dense_mlp).

Reference computation:
    sized = broadcast(representation[B,1,R] -> [B,S,R])   (ones @ rep)
    h     = relu(sized @ W1^T + b1)                       [B,S,HID]
    out   = h @ W2^T + b2                                 [B,S,OUT]

Because every position s within batch b receives the identical input row
representation[b], the MLP output row is identical for all S positions:
    row[b] = relu(rep[b] @ W1^T + b1) @ W2^T + b2         [B,OUT]
    out[b, s, :] = row[b]  for all s

Sharding: the S axis is degenerate, so the device only computes the
unique rows. OUT columns are sharded 8 ways: every core computes all
B=32 batch rows for its own 128-column slice of the output, writing a
[32,128] f32 shard. The host unshards by concatenating the column
slices and broadcasting the rows across S.

This makes the kernel input-DMA-bound: W1 (replicated, needed in full
by every core because every core computes h for all batches) dominates.
Weights/activations are staged in bf16 (halves DMA bytes; rel-err
~3e-3, far inside the 2e-2 gate); PSUM accumulation stays fp32.

Device pipeline per core:
  1. ~3.4 us of contiguous dummy matmuls on zeros warm the PE HAM
     clock gate (1.2 -> 2.4 GHz) while the weights stream in; the
     last one is gated on the first input DMA. The warm transition
     needs a ~3.4us UNBROKEN busy streak — shorter warmups leave the
     whole kernel cold at 2x matmul cost.
  2. The four K=1 ones-matmul bias terms (b1 halves into ph_a/ph_b,
     b2 halves into py_l/py_r) open their PSUM accumulation groups
     early, off the critical tail.
  3. Two half-pipelines over h-columns, streamed as four 256KB w1
     quarters on the sync ring (quarter 0 merged into the pk DMA so
     L1's first group is ready ~1.4us earlier): per half g, 8
     accumulating N=256 matmuls (x^T chunks stationary, cheap
     LDWEIGHTS), DVE relu+bf16-cast, 2 PE transposes H->H^T (bf16
     PSUM) + DVE copies, then the half's two L2 accumulations into
     each output-column half. Half 0's transposes/L2 run while half
     1 still streams.
  4. L2 finishes per output-column half so py_l's PSUM->SBUF copy
     (DVE) and 8 KiB output DMA (sync ring) overlap py_r's matmuls,
     whose copy (ACT) + DMA ride the scalar ring.

Single-sync-wait discipline (walrus rejects 2+ waits per instruction):
the last warmup matmul pre-observes pk's DMA lane (transposes read the
identity from pk), transpose #1 pre-observes w2s's lane for L2, biases
ride prow's lane once, separate PSUM tiles per half avoid Tile's
non-elidable co-reader/WAR serialization, and a chain of 1-wait SP
nops before the TileContext exit drain leaves the drain with nothing
to wait on. A nosync ordering edge keeps half 0's ready transposes/L2
matmuls ahead of the w1B-gated L1b in the PE queue.

HW-measured notes that shaped the DMA layout: per-partition descriptor
size dominates HBM-read rate (576B -> ~76 GB/s, 2KB -> ~200, 4KB ->
~220, 8KB -> ~260); extra queues do NOT add aggregate bandwidth (8
cores share HBM); the scalar HWDGE ring starts streaming ~2us after
sync and the SWDGE (gpsimd) queue has ~3-4us first-data latency, so
bulk rides sync and only small/late-needed tensors ride scalar. The PE
HAM clock gate re-throttles after ~2.5us of PE idle, halving matmul
throughput — warmup length and DMA/compute interleave are chosen to
keep PE continuously busy from warmup through the tail.
"""

import sys

import numpy as np

if "/opt/trn_rl_repo" not in sys.path:
    sys.path.insert(0, "/opt/trn_rl_repo")

import ml_dtypes

BF16 = ml_dtypes.bfloat16

B, S, R = 32, 1024, 1024
HID, OUT = 512, 1024
N_CORES = 8
OSL = OUT // N_CORES  # output columns per core

RC = R // 128  # layer-1 contraction chunks
HC = HID // 128  # layer-2 contraction chunks

# pk columns: xT chunks [p, rc*B + m] = rep[m, rc*128+p], then a 32x32
# identity for the PE transposes
XTOFF = 0
IOFF = XTOFF + RC * B
PKW = IOFF + B
# prow columns (single partition row): ones, b1, b2 slice
ONOFF = 0
B1OFF = ONOFF + B
B2OFF = B1OFF + HID
PROWW = B2OFF + OSL

N_WARMUP = 8
N_STARTER = 5

_CACHED_NC = None


def _build_nc():
    import concourse.bass as bass
    import concourse.mybir as mybir
    from concourse.tile import TileContext, add_dep_helper

    f32 = mybir.dt.float32
    bf16 = mybir.dt.bfloat16
    fcopy = mybir.ActivationFunctionType.Copy
    nc = bass.Bass()

    QW = RC * HID // 4
    pkq0 = nc.dram_tensor("pkq0", [128, PKW + QW], bf16, kind="ExternalInput")
    prow = nc.dram_tensor("prow", [1, PROWW], bf16, kind="ExternalInput")
    w1r = nc.dram_tensor("w1r", [128, 3 * QW], bf16, kind="ExternalInput")
    w2s = nc.dram_tensor("w2s", [128, HC * OSL], bf16, kind="ExternalInput")
    out = nc.dram_tensor("out", [B, OSL], f32, kind="ExternalOutput")

    with TileContext(nc) as tc:
        with (
            tc.tile_pool(name="const", bufs=1) as cpool,
            tc.tile_pool(name="psum_s", bufs=1, space="PSUM") as pp_s,
            tc.tile_pool(name="psum_t", bufs=2, space="PSUM") as pp_t,
            tc.tile_pool(name="psum_y", bufs=1, space="PSUM") as pp_y,
        ):
            # Sync ring: one merged DMA carrying pk + w1 quarter 0 (gates
            # the warmup tail AND L1a's first four matmuls together at
            # ~10.8us), then w1 quarters 1-3 so each L1 matmul group starts
            # as soon as its 256KB quarter lands. Scalar ring (starts ~2us
            # later): prow then w2s, both needed later. See module
            # docstring for the measured DMA behavior behind this layout.
            qw = RC * HID // 4
            pkq0_sb = cpool.tile([128, PKW + qw], bf16, tag="pkq0")
            d_pkq0 = nc.sync.dma_start(out=pkq0_sb[:, :], in_=pkq0[:, :])
            w1r_sb = cpool.tile([128, 3 * qw], bf16, tag="w1r")
            d_q1 = nc.sync.dma_start(out=w1r_sb[:, 0:qw], in_=w1r[:, 0:qw])
            d_q2 = nc.sync.dma_start(
                out=w1r_sb[:, qw : 2 * qw], in_=w1r[:, qw : 2 * qw]
            )
            d_q3 = nc.sync.dma_start(
                out=w1r_sb[:, 2 * qw : 3 * qw], in_=w1r[:, 2 * qw : 3 * qw]
            )
            prow_sb = cpool.tile([1, PROWW], bf16, tag="prow")
            d_prow = nc.scalar.dma_start(out=prow_sb[0:1, :], in_=prow[0:1, :])
            w2s_sb = cpool.tile([128, HC * OSL], bf16, tag="w2s")
            d_w2s = nc.scalar.dma_start(out=w2s_sb[:, :], in_=w2s[:, :])
            d_w1 = [d_q1, d_q2, d_q3]
            d_pk = d_pkq0
            pk_sb = pkq0_sb

            # ---- PE warmup on zeros; shares L1's PSUM tile (a slot handoff
            # would emit a non-elidable same-engine wait) -------------------
            # Hybrid warmup: a tiny [128,128] memset un-gates N=128 starter
            # matmuls ~1us before the full memset could (streak begins
            # ~7.4), the second memset fills the rest of the tile under
            # them, then N=512 warmups take over — HW-measured, the HAM
            # warm transition fires ~2.5-3us into an N=512 streak but only
            # ~6us into an N=128 one (narrow matmuls count less toward
            # PE-busy), so the wide phase does the actual warming and the
            # whole streak ends ~1us earlier than all-wide-from-memset.
            wm_sb = cpool.tile([128, 512], bf16, tag="wm")
            nc.vector.memset(wm_sb[:, 0:128], 0.0)
            ph_full = pp_s.tile([128, HID], f32, tag="s")
            for k in range(N_STARTER):
                nc.tensor.matmul(
                    ph_full[:, 0:128],
                    lhsT=wm_sb[:, 0:128],
                    rhs=wm_sb[:, 0:128],
                    start=True,
                    stop=True,
                )
            nc.vector.memset(wm_sb[:, 128:512], 0.0)
            for k in range(N_WARMUP):
                wmm = nc.tensor.matmul(
                    ph_full[:, :],
                    lhsT=wm_sb[:, 0:128],
                    rhs=wm_sb[:, :],
                    start=True,
                    stop=True,
                )
            # the last warmup matmul observes pk's lane so L1's first matmul
            # only needs the w1-chunk-0 wait
            add_dep_helper(wmm.ins, d_pk.ins, sync=True, reason="observe pk")

            # All four bias matmuls run up front as the accumulation-group
            # openers (start=True), gated only on prow which lands early —
            # this takes ~0.5us of K=1 matmuls off the serial tail.

            # ---- Two half-pipelines over h-columns. w1 is packed so half g
            # holds W1 rows [g*256, (g+1)*256) for every rc chunk; half 0's
            # L1 matmuls, relu, transposes and first two L2 accumulations all
            # run while half 1 is still streaming. --------------------------
            # separate PSUM tiles per half — co-readers/WAR on one shared
            # PSUM tile get serialized by Tile with non-elidable waits.
            # L2 accumulates into two output-column halves so the first
            # half's PSUM->SBUF copy and output DMA overlap the second
            # half's matmuls (and the two 8KB DMAs ride separate rings).
            HH = HID // 2
            OHL = OSL // 2
            ht_sb = cpool.tile([128, HC * B], bf16, tag="ht")
            py_l = pp_y.tile([B, OHL], f32, tag="yl")
            py_r = pp_y.tile([B, OHL], f32, tag="yr")
            ph_a = pp_s.tile([B, HH], f32, tag="pha")
            ph_b = pp_s.tile([B, HH], f32, tag="phb")
            ph_halves = [ph_a, ph_b]
            ones = prow_sb[0:1, ONOFF : ONOFF + B]
            for g in range(2):
                nc.tensor.matmul(
                    ph_halves[g][:, :],
                    lhsT=ones,
                    rhs=prow_sb[0:1, B1OFF + g * HH : B1OFF + (g + 1) * HH],
                    start=True,
                    stop=False,
                )
            nc.tensor.matmul(
                py_l[:, :],
                lhsT=ones,
                rhs=prow_sb[0:1, B2OFF : B2OFF + OHL],
                start=True,
                stop=False,
            )
            nc.tensor.matmul(
                py_r[:, :],
                lhsT=ones,
                rhs=prow_sb[0:1, B2OFF + OHL : B2OFF + OSL],
                start=True,
                stop=False,
            )
            g0_last_pe = None
            for g in range(2):
                ph_g = ph_halves[g]
                for rc in range(RC):
                    if g == 0 and rc < RC // 2:
                        w1rhs = pkq0_sb[:, PKW + rc * HH : PKW + (rc + 1) * HH]
                    elif g == 0:
                        w1rhs = w1r_sb[:, (rc - RC // 2) * HH : (rc - RC // 2 + 1) * HH]
                    else:
                        w1rhs = w1r_sb[:, qw + rc * HH : qw + (rc + 1) * HH]
                    l1mm = nc.tensor.matmul(
                        ph_g[:, :],
                        lhsT=pk_sb[:, XTOFF + rc * B : XTOFF + (rc + 1) * B],
                        rhs=w1rhs,
                        start=False,
                        stop=(rc == RC - 1),
                    )
                    if g == 1 and rc == 0 and g0_last_pe is not None:
                        # ordering-only dep: keep half 0's ready transposes /
                        # L2 matmuls ahead of the w1B-gated L1b in the PE
                        # queue (the scheduler's DMA model otherwise hoists
                        # L1b first and it head-of-line blocks the engine)
                        add_dep_helper(
                            l1mm.ins,
                            g0_last_pe.ins,
                            sync=False,
                            reason="T/L2 before w1B-gated L1b",
                        )
                # single relu pass per half: splitting it into two [32,128]
                # passes (so T0 waits only the first) measured WORSE — the
                # ~200ns DVE fixed cost per op exceeds the earlier handoff
                h_g = cpool.tile([B, HH], bf16, tag=f"h{g}")
                nc.vector.tensor_scalar_max(h_g[:, :], ph_g[:, :], 0.0)
                for j in range(2):
                    hc = g * 2 + j
                    pt = pp_t.tile([128, B], bf16, tag="t")
                    tmm = nc.tensor.transpose(
                        pt[:, :],
                        h_g[0:B, j * 128 : (j + 1) * 128],
                        pk_sb[0:B, IOFF : IOFF + B],
                    )
                    if hc == 1:
                        # free wait slot: pre-observe w2s's lane for L2
                        add_dep_helper(
                            tmm.ins, d_w2s.ins, sync=True, reason="observe w2s"
                        )
                    dst = ht_sb[:, hc * B : (hc + 1) * B]
                    nc.vector.tensor_copy(dst, pt[:, :])
                for j in range(2):
                    hc = g * 2 + j
                    nc.tensor.matmul(
                        py_l[:, :],
                        lhsT=ht_sb[:, hc * B : (hc + 1) * B],
                        rhs=w2s_sb[:, hc * OSL : hc * OSL + OHL],
                        start=False,
                        stop=(hc == HC - 1),
                    )
                for j in range(2):
                    hc = g * 2 + j
                    last_mm = nc.tensor.matmul(
                        py_r[:, :],
                        lhsT=ht_sb[:, hc * B : (hc + 1) * B],
                        rhs=w2s_sb[:, hc * OSL + OHL : (hc + 1) * OSL],
                        start=False,
                        stop=(hc == HC - 1),
                    )
                g0_last_pe = last_mm
            # the r half finishes LAST, so it gets the faster DVE copy
            # (~211ns vs ACT's ~305); the earlier l half absorbs ACT's cost
            o_l = cpool.tile([B, OHL], f32, tag="ol")
            o_r = cpool.tile([B, OHL], f32, tag="or")
            last_act = nc.scalar.activation(o_l[:, :], py_l[:, :], fcopy)
            d_out_l = nc.sync.dma_start(out=out[:, 0:OHL], in_=o_l[:, :])
            last_dve = nc.vector.tensor_copy(o_r[:, :], py_r[:, :])
            d_out_r = nc.scalar.dma_start(out=out[:, OHL:OSL], in_=o_r[:, :])

            # The kernel-tail drain waits on every proc's final tick, but this
            # walrus allows at most ONE sync wait per instruction. Chain SP
            # nops, one dependency each, so SP's vector clock observes the
            # final tick of every DMA lane and engine before the drain.
            # latest-resolving deps (the output DMAs) go LAST: the nops run
            # serially on SP, so anything after a blocked nop retires after
            # it — early-resolved input/engine nops must not queue behind
            # the out-sems that fire at the very end of the kernel
            tail = [d_pk, d_prow] + d_w1 + [
                d_w2s,
                last_mm,
                last_act,
                last_dve,
                d_out_l,
                d_out_r,
            ]
            for d in tail:
                n = nc.sync.nop(nofuse=True)
                add_dep_helper(
                    n.ins, d.ins, sync=True, reason="observe final ticks pre-drain"
                )

    return nc


def _get_nc():
    global _CACHED_NC
    if _CACHED_NC is None:
        _CACHED_NC = _build_nc()
    return _CACHED_NC


def _prep_in_maps(representation, W1, b1, W2, b2):
    rep = np.asarray(representation, dtype=np.float32).reshape(B, R)
    w1 = np.asarray(W1, dtype=np.float32)
    w2 = np.asarray(W2, dtype=np.float32)
    b1 = np.asarray(b1, dtype=np.float32)
    b2 = np.asarray(b2, dtype=np.float32)

    # pk: xT chunks + 32x32 identity (identical for every core)
    pk = np.zeros((128, PKW), dtype=np.float32)
    xt = rep.T  # [R, B]
    pk[:, XTOFF : XTOFF + RC * B] = (
        xt.reshape(RC, 128, B).transpose(1, 0, 2).reshape(128, RC * B)
    )
    pk[0:B, IOFF : IOFF + B] = np.eye(B, dtype=np.float32)
    pk = pk.astype(BF16)

    # w1p[p, g*2048 + rc*256 + h'] = W1[g*256 + h', rc*128+p] — h-half-major
    # so each 256KB DMA quarter covers 4 rc chunks for one 256-column half.
    # Quarter 0 is packed together with pk into one DMA; quarters 1-3 form
    # the w1r tensor.
    w1p = np.ascontiguousarray(
        w1.T.reshape(RC, 128, 2, HID // 2)
        .transpose(1, 2, 0, 3)
        .reshape(128, RC * HID)
    ).astype(BF16)
    qw = RC * HID // 4
    pkq0 = np.concatenate([pk, w1p[:, 0:qw]], axis=1)
    w1r = np.ascontiguousarray(w1p[:, qw:])

    in_maps = []
    for c in range(N_CORES):
        sl = slice(c * OSL, (c + 1) * OSL)
        prow = np.zeros((1, PROWW), dtype=np.float32)
        prow[0, ONOFF : ONOFF + B] = 1.0
        prow[0, B1OFF : B1OFF + HID] = b1
        prow[0, B2OFF : B2OFF + OSL] = b2[sl]
        # w2sp[p, hc*OSL + o] = W2[c*OSL+o, hc*128+p]
        w2sl = w2[sl]  # [OSL, HID]
        w2sp = np.ascontiguousarray(
            w2sl.T.reshape(HC, 128, OSL).transpose(1, 0, 2).reshape(128, HC * OSL)
        ).astype(BF16)
        in_maps.append(
            {"pkq0": pkq0, "prow": prow.astype(BF16), "w1r": w1r, "w2s": w2sp}
        )
    return in_maps


def run_sharded(representation, W1, b1, W2, b2, **run_kwargs):
    """Compile+run on 8 cores; returns (full_output, BassKernelResults)."""
    from concourse.bass_utils import run_bass_kernel_spmd

    nc = _get_nc()
    in_maps = _prep_in_maps(representation, W1, b1, W2, b2)
    res = run_bass_kernel_spmd(nc, in_maps, core_ids=list(range(N_CORES)), **run_kwargs)
    rows = np.concatenate([r["out"] for r in res.results], axis=1)  # [B, OUT]
    full = np.ascontiguousarray(
        np.broadcast_to(rows[:, None, :], (B, S, OUT))
    )
    return full, res


def kernel(representation, size_matrix=None, W1=None, b1=None, W2=None, b2=None):
    # size_matrix only contributes its shape in the reference (ones_like);
    # its values are unused.
    full, _ = run_sharded(representation, W1, b1, W2, b2)
    return full
